# revision 24
# baseline (speedup 1.0000x reference)
"""Trainium2 Bass kernel for the coverage-attention module.

Computes, for inputs h_c_hat [B,N], enc_outputs [B,M,N], enc_feature [B*M,N],
enc_padding_mask [B,M], coverage [B,M] and params Wd [N,N], bd, wc, v [N]:

    dec      = h_c_hat @ Wd.T + bd                      # [B,N]
    att      = enc_feature + dec[:,None,:] + coverage[:,:,None]*wc
    scores   = tanh(att) @ v                            # [B,M]
    attn     = softmax(scores) * (1-mask); attn /= attn.sum(-1)
    context  = einsum('bm,bmn->bn', attn, enc_outputs)
    new_cov  = coverage + attn

Sharding: data-parallel over batch across 8 NeuronCores (4 batches/core),
params replicated. Inside each core everything stays in the natural
[m-on-partitions, n-on-free] layout:
  * rank-2 term ones*dec + cov*wc is a K=2 matmul accumulated in PSUM on top
    of an identity-matmul copy of enc_feature (float32r, 1 cyc/row),
  * tanh on ScalarE reads PSUM directly,
  * scores' n-contraction is the fused DVE tensor_tensor_reduce (accum_out),
  * softmax's m-reduction is a tiny GPSIMD partition reduce,
  * context matmul contracts m on partitions (natural layout).
Host-side prep is layout-only: Wd and h_c_hat are passed pre-transposed.
"""

import os

import numpy as np

B, M, N = 32, 2048, 1024
NCORES = 8
BL = B // NCORES          # batches per core
P = 128                   # partitions
MT = M // P               # m-tiles per batch
MG_F = 8                  # m-tiles per enc_feature DMA group
MG_O = 8                  # m-tiles per enc_outputs DMA group
NHALF = N // 512

_CACHE = {}
LAST_RESULT = None        # BassKernelResults of the most recent run (for test.py)

# float32r (single-pass reduced-precision fp32 matmul) for the bulk matmuls;
# flip to exact fp32 (4 cyc/row) if precision ever demands it.
USE_F32R_ENC = True
USE_F32R_CTX = True


def _build_nc():
    import concourse.bass as bass
    import concourse.tile as tile
    from concourse import bacc, bass_isa, library_config, mybir

    f32 = mybir.dt.float32
    f32r = mybir.dt.float32r
    Alu = mybir.AluOpType
    Act = mybir.ActivationFunctionType

    enc_dt = f32r if USE_F32R_ENC else f32
    ctx_dt = f32r if USE_F32R_CTX else f32

    nc = bacc.Bacc("TRN2", target_bir_lowering=False, debug=False)

    encf = nc.dram_tensor("enc_feature", [BL * M, N], enc_dt, kind="ExternalInput").ap()
    enco = nc.dram_tensor("enc_outputs", [BL, M, N], ctx_dt, kind="ExternalInput").ap()
    hT = nc.dram_tensor("h_T", [N, BL], enc_dt, kind="ExternalInput").ap()
    WdT = nc.dram_tensor("WdT", [N, N], enc_dt, kind="ExternalInput").ap()
    bd = nc.dram_tensor("bd", [N], f32, kind="ExternalInput").ap()
    wc = nc.dram_tensor("wc", [N], enc_dt, kind="ExternalInput").ap()
    v = nc.dram_tensor("v", [N], f32, kind="ExternalInput").ap()
    cov = nc.dram_tensor("coverage", [BL, M], enc_dt, kind="ExternalInput").ap()
    mask = nc.dram_tensor("mask", [BL, M], f32, kind="ExternalInput").ap()
    ones = nc.dram_tensor("ones", [M], enc_dt, kind="ExternalInput").ap()
    ctx_out = nc.dram_tensor("context_vec", [BL, N], f32, kind="ExternalOutput").ap()
    attn_out = nc.dram_tensor("attn", [BL, M], f32, kind="ExternalOutput").ap()
    ncov_out = nc.dram_tensor("new_coverage", [BL, M], f32, kind="ExternalOutput").ap()

    def pbcast(ap, parts):
        # 0-stride partition broadcast of a 1-partition source AP (DRAM src only)
        return bass.AP(tensor=ap.tensor, offset=ap.offset, ap=[[0, parts], *ap.ap])

    with tile.TileContext(nc) as tc:
        consts = tc.alloc_tile_pool(name="consts", bufs=1)
        apool = tc.alloc_tile_pool(name="apool", bufs=3, space="PSUM")
        cpool = tc.alloc_tile_pool(name="cpool", bufs=1, space="PSUM")
        setup = tc.alloc_tile_pool(name="setup", bufs=1)

        nc.gpsimd.load_library(library_config.attn)

        v_bc = consts.tile([P, N], f32)
        nc.gpsimd.dma_start(out=v_bc, in_=pbcast(v.unsqueeze(0), P))

        # ---- dec = h @ Wd.T + bd, full fp32 (setup; WdT freed afterwards) ----
        WdT_sb = setup.tile([P, N // P, N], enc_dt)
        WdT_r = WdT.rearrange("(c p) i -> p c i", p=P)
        hT_sb = setup.tile([P, N // P, BL], enc_dt)
        nc.scalar.dma_start(out=hT_sb, in_=hT.rearrange("(c p) b -> p c b", p=P))
        bd_bc = setup.tile([BL, N], f32)
        nc.gpsimd.dma_start(out=bd_bc, in_=pbcast(bd.unsqueeze(0), BL))

        dec_ps = cpool.tile([BL, N], f32, tag="ctx")
        for jc in range(N // P):
            nc.sync.dma_start(out=WdT_sb[:, jc, :], in_=WdT_r[:, jc, :])
            for h in range(NHALF):
                nc.tensor.matmul(
                    dec_ps[:, h * 512:(h + 1) * 512],
                    lhsT=hT_sb[:, jc, :],
                    rhs=WdT_sb[:, jc, h * 512:(h + 1) * 512],
                    start=(jc == 0),
                    stop=(jc == N // P - 1),
                )
        dec_sb = consts.tile([BL, N], enc_dt)
        nc.vector.tensor_add(dec_sb, dec_ps, bd_bc)
        setup.release()

        fpool = tc.alloc_tile_pool(name="fpool", bufs=2)
        opool = tc.alloc_tile_pool(name="opool", bufs=2)
        tpool = tc.alloc_tile_pool(name="tpool", bufs=3)
        spool = tc.alloc_tile_pool(name="spool", bufs=2)
        uvpool = tc.alloc_tile_pool(name="uvpool", bufs=2)

        # m index mapping everywhere: m = p * MT + t  (p on partitions)
        for b in range(BL):
            # rank-2 operands: U_b = [ones; coverage[b]] (strided views per
            # tile), V_b = [dec[b]; wc]
            U_b = uvpool.tile([2, M], enc_dt, name="U")
            nc.gpsimd.dma_start(out=U_b[0:1, :], in_=ones.unsqueeze(0))
            nc.gpsimd.dma_start(out=U_b[1:2, :], in_=cov[b:b + 1, :])
            Uv = U_b.rearrange("k (p t) -> k t p", t=MT)
            V_b = uvpool.tile([2, N], enc_dt, name="V")
            nc.gpsimd.dma_start(out=V_b[1:2, :], in_=wc.unsqueeze(0))
            nc.gpsimd.dma_start(out=V_b[0:1, :], in_=dec_sb[b:b + 1, :])

            # phase 1: att -> tanh -> scores
            scores_b = spool.tile([P, MT], f32, name="scores")
            for g in range(MT // MG_F):
                encf_t = fpool.tile([P, MG_F, N], enc_dt)
                src = encf[b * M:(b + 1) * M, :].rearrange(
                    "(p t) n -> p t n", t=MT)[:, g * MG_F:(g + 1) * MG_F, :]
                nc.sync.dma_start(out=encf_t, in_=src)
                for t in range(MG_F):
                    mt = g * MG_F + t
                    att_ps = apool.tile([P, N], f32, name="att")
                    for h in range(NHALF):
                        sl = slice(h * 512, (h + 1) * 512)
                        nc.tensor.matmul(
                            att_ps[:, sl], lhsT=Uv[:, mt, :], rhs=V_b[:, sl],
                            start=True, stop=True,
                        )
                    att_sl = encf_t[:, t, :].bitcast(f32)
                    nc.vector.scalar_tensor_tensor(
                        out=att_sl, in0=att_sl, scalar=1.0, in1=att_ps,
                        op0=Alu.mult, op1=Alu.add,
                    )
                    tanh_t = tpool.tile([P, N], f32, name="tanh")
                    nc.scalar.activation(tanh_t, att_sl, Act.Tanh)
                    nc.vector.scalar_tensor_tensor(
                        out=tanh_t, in0=tanh_t, scalar=1.0, in1=v_bc,
                        op0=Alu.mult, op1=Alu.mult,
                        accum_out=scores_b[:, mt:mt + 1],
                    )

            # phase 2: masked softmax over m
            exp_t = spool.tile([P, MT], f32, name="exp")
            nc.scalar.activation(exp_t, scores_b, Act.Exp)
            mask_t = spool.tile([P, MT], f32, name="mask_t")
            nc.gpsimd.dma_start(out=mask_t, in_=mask[b, :].rearrange("(p t) -> p t", t=MT))
            om_t = spool.tile([P, MT], f32, name="om")
            nc.vector.tensor_scalar(om_t, mask_t, -1.0, 1.0, Alu.mult, Alu.add)
            rsum = spool.tile([P, 1], f32, name="rsum")
            me_t = spool.tile([P, MT], f32, name="me")
            nc.vector.scalar_tensor_tensor(
                out=me_t, in0=exp_t, scalar=1.0, in1=om_t,
                op0=Alu.mult, op1=Alu.mult, accum_out=rsum,
            )
            tot_b = spool.tile([P, 1], f32, name="tot_b")
            nc.gpsimd.partition_all_reduce(
                tot_b, rsum, channels=P, reduce_op=bass_isa.ReduceOp.add
            )
            rb = spool.tile([P, 1], f32, name="rb")
            nc.vector.reciprocal(rb, tot_b)
            attn_t = spool.tile([P, MT], f32, name="attn_t")
            nc.vector.tensor_scalar_mul(attn_t, me_t, rb)
            cov_t = spool.tile([P, MT], enc_dt, name="cov_t")
            nc.gpsimd.dma_start(out=cov_t, in_=cov[b, :].rearrange("(p t) -> p t", t=MT))
            ncov_t = spool.tile([P, MT], f32, name="ncov_t")
            nc.vector.tensor_add(ncov_t, cov_t, attn_t)
            nc.gpsimd.dma_start(
                out=attn_out[b, :].rearrange("(p t) -> p t", t=MT), in_=attn_t
            )
            nc.gpsimd.dma_start(
                out=ncov_out[b, :].rearrange("(p t) -> p t", t=MT), in_=ncov_t
            )

            # phase 3: context = attn @ enc_outputs[b]
            attn_r = spool.tile([P, MT], ctx_dt, name="attn_r")
            nc.vector.tensor_copy(attn_r, attn_t)
            ctx_ps = cpool.tile([1, N], f32, tag="ctx", name="ctx_ps")
            for g in range(MT // MG_O):
                enco_t = opool.tile([P, MG_O, N], ctx_dt)
                src = enco[b, :, :].rearrange(
                    "(p t) n -> p t n", t=MT)[:, g * MG_O:(g + 1) * MG_O, :]
                nc.scalar.dma_start(out=enco_t, in_=src)
                for t in range(MG_O):
                    mt = g * MG_O + t
                    for h in range(NHALF):
                        sl = slice(h * 512, (h + 1) * 512)
                        nc.tensor.matmul(
                            ctx_ps[0:1, sl],
                            lhsT=attn_r[:, mt:mt + 1],
                            rhs=enco_t[:, t, sl],
                            start=(mt == 0),
                            stop=(mt == MT - 1),
                        )
            ctx_sb = spool.tile([1, N], f32, name="ctx_sb")
            nc.scalar.copy(ctx_sb, ctx_ps)
            nc.gpsimd.dma_start(out=ctx_out[b:b + 1, :], in_=ctx_sb)

        uvpool.release()
        spool.release()
        tpool.release()
        opool.release()
        fpool.release()
        cpool.release()
        apool.release()
        consts.release()

    nc.finalize()
    return nc


def kernel(h_c_hat, enc_outputs, enc_feature, enc_padding_mask, coverage,
           Wd, bd, wc, v):
    global LAST_RESULT
    from concourse.bass_utils import run_bass_kernel_spmd

    if "nc" not in _CACHE:
        _CACHE["nc"] = _build_nc()
    nc = _CACHE["nc"]

    f = np.float32
    WdT = np.ascontiguousarray(np.asarray(Wd, dtype=f).T)
    hT_full = np.ascontiguousarray(np.asarray(h_c_hat, dtype=f).T)  # [N, B]
    enc_feature = np.asarray(enc_feature, dtype=f).reshape(B, M, N)
    enc_outputs = np.asarray(enc_outputs, dtype=f)
    enc_padding_mask = np.asarray(enc_padding_mask, dtype=f)
    coverage = np.asarray(coverage, dtype=f)
    bd = np.ascontiguousarray(np.asarray(bd, dtype=f))
    wc = np.ascontiguousarray(np.asarray(wc, dtype=f))
    v = np.ascontiguousarray(np.asarray(v, dtype=f))

    in_maps = []
    for c in range(NCORES):
        s = slice(c * BL, (c + 1) * BL)
        in_maps.append({
            "enc_feature": np.ascontiguousarray(
                enc_feature[s].reshape(BL * M, N)),
            "enc_outputs": np.ascontiguousarray(enc_outputs[s]),
            "h_T": np.ascontiguousarray(hT_full[:, s]),
            "WdT": WdT,
            "bd": bd,
            "wc": wc,
            "v": v,
            "coverage": np.ascontiguousarray(coverage[s]),
            "mask": np.ascontiguousarray(enc_padding_mask[s]),
            "ones": np.ones(M, dtype=f),
        })

    trace = bool(os.environ.get("BASS_TRACE"))
    tmpdir = os.environ.get("BASS_KERNEL_TMPDIR") or None
    LAST_RESULT = run_bass_kernel_spmd(
        nc, in_maps, core_ids=list(range(NCORES)), trace=trace, tmpdir=tmpdir
    )
    res = LAST_RESULT.results

    context_vec = np.concatenate([res[c]["context_vec"] for c in range(NCORES)], 0)
    attn = np.concatenate([res[c]["attn"] for c in range(NCORES)], 0)
    new_coverage = np.concatenate([res[c]["new_coverage"] for c in range(NCORES)], 0)
    return context_vec, attn, new_coverage


# revision 25
# speedup vs baseline: 1.0483x; 1.0483x over previous
"""Trainium2 Bass kernel for the coverage-attention module.

Computes, for inputs h_c_hat [B,N], enc_outputs [B,M,N], enc_feature [B*M,N],
enc_padding_mask [B,M], coverage [B,M] and params Wd [N,N], bd, wc, v [N]:

    dec      = h_c_hat @ Wd.T + bd                      # [B,N]
    att      = enc_feature + dec[:,None,:] + coverage[:,:,None]*wc
    scores   = tanh(att) @ v                            # [B,M]
    attn     = softmax(scores) * (1-mask); attn /= attn.sum(-1)
    context  = einsum('bm,bmn->bn', attn, enc_outputs)
    new_cov  = coverage + attn

Sharding: data-parallel over batch across 8 NeuronCores (4 batches/core),
params replicated. Inside each core everything stays in the natural
[m-on-partitions, n-on-free] layout:
  * rank-2 term ones*dec + cov*wc is a K=2 matmul accumulated in PSUM on top
    of an identity-matmul copy of enc_feature (float32r, 1 cyc/row),
  * tanh on ScalarE reads PSUM directly,
  * scores' n-contraction is the fused DVE tensor_tensor_reduce (accum_out),
  * softmax's m-reduction is a tiny GPSIMD partition reduce,
  * context matmul contracts m on partitions (natural layout).
Host-side prep is layout-only: Wd and h_c_hat are passed pre-transposed.
"""

import os

import numpy as np

B, M, N = 32, 2048, 1024
NCORES = 8
BL = B // NCORES          # batches per core
P = 128                   # partitions
MT = M // P               # m-tiles per batch
MG_F = 4                  # m-tiles per enc_feature DMA group
MG_O = 4                  # m-tiles per enc_outputs DMA group
NHALF = N // 512

_CACHE = {}
LAST_RESULT = None        # BassKernelResults of the most recent run (for test.py)

# float32r (single-pass reduced-precision fp32 matmul) for the bulk matmuls;
# flip to exact fp32 (4 cyc/row) if precision ever demands it.
USE_F32R_ENC = True
USE_F32R_CTX = True


def _build_nc():
    import concourse.bass as bass
    import concourse.tile as tile
    from concourse import bacc, bass_isa, library_config, mybir

    f32 = mybir.dt.float32
    f32r = mybir.dt.float32r
    Alu = mybir.AluOpType
    Act = mybir.ActivationFunctionType

    enc_dt = f32r if USE_F32R_ENC else f32
    ctx_dt = f32r if USE_F32R_CTX else f32

    nc = bacc.Bacc("TRN2", target_bir_lowering=False, debug=False)

    encf = nc.dram_tensor("enc_feature", [BL * M, N], enc_dt, kind="ExternalInput").ap()
    enco = nc.dram_tensor("enc_outputs", [BL, M, N], ctx_dt, kind="ExternalInput").ap()
    hT = nc.dram_tensor("h_T", [N, BL], enc_dt, kind="ExternalInput").ap()
    WdT = nc.dram_tensor("WdT", [N, N], enc_dt, kind="ExternalInput").ap()
    bd = nc.dram_tensor("bd", [N], f32, kind="ExternalInput").ap()
    wc = nc.dram_tensor("wc", [N], enc_dt, kind="ExternalInput").ap()
    v = nc.dram_tensor("v", [N], f32, kind="ExternalInput").ap()
    cov = nc.dram_tensor("coverage", [BL, M], enc_dt, kind="ExternalInput").ap()
    mask = nc.dram_tensor("mask", [BL, M], f32, kind="ExternalInput").ap()
    ones = nc.dram_tensor("ones", [M], enc_dt, kind="ExternalInput").ap()
    ctx_out = nc.dram_tensor("context_vec", [BL, N], f32, kind="ExternalOutput").ap()
    attn_out = nc.dram_tensor("attn", [BL, M], f32, kind="ExternalOutput").ap()
    ncov_out = nc.dram_tensor("new_coverage", [BL, M], f32, kind="ExternalOutput").ap()

    def pbcast(ap, parts):
        # 0-stride partition broadcast of a 1-partition source AP (DRAM src only)
        return bass.AP(tensor=ap.tensor, offset=ap.offset, ap=[[0, parts], *ap.ap])

    with tile.TileContext(nc) as tc:
        consts = tc.alloc_tile_pool(name="consts", bufs=1)
        apool = tc.alloc_tile_pool(name="apool", bufs=3, space="PSUM")
        cpool = tc.alloc_tile_pool(name="cpool", bufs=1, space="PSUM")
        setup = tc.alloc_tile_pool(name="setup", bufs=1)

        nc.gpsimd.load_library(library_config.attn)

        v_bc = consts.tile([P, N], f32)
        nc.gpsimd.dma_start(out=v_bc, in_=pbcast(v.unsqueeze(0), P))

        # ---- dec = h @ Wd.T + bd, full fp32 (setup; WdT freed afterwards) ----
        WdT_sb = setup.tile([P, N // P, N], enc_dt)
        WdT_r = WdT.rearrange("(c p) i -> p c i", p=P)
        hT_sb = setup.tile([P, N // P, BL], enc_dt)
        nc.scalar.dma_start(out=hT_sb, in_=hT.rearrange("(c p) b -> p c b", p=P))
        bd_bc = setup.tile([BL, N], f32)
        nc.gpsimd.dma_start(out=bd_bc, in_=pbcast(bd.unsqueeze(0), BL))

        dec_ps = cpool.tile([BL, N], f32, tag="ctx")
        for jc in range(N // P):
            nc.scalar.dma_start(out=WdT_sb[:, jc, :], in_=WdT_r[:, jc, :])
            for h in range(NHALF):
                nc.tensor.matmul(
                    dec_ps[:, h * 512:(h + 1) * 512],
                    lhsT=hT_sb[:, jc, :],
                    rhs=WdT_sb[:, jc, h * 512:(h + 1) * 512],
                    start=(jc == 0),
                    stop=(jc == N // P - 1),
                )
        dec_sb = consts.tile([BL, N], enc_dt)
        nc.vector.tensor_add(dec_sb, dec_ps, bd_bc)
        setup.release()

        fpool = tc.alloc_tile_pool(name="fpool", bufs=4)
        opool = tc.alloc_tile_pool(name="opool", bufs=4)
        tpool = tc.alloc_tile_pool(name="tpool", bufs=3)
        spool = tc.alloc_tile_pool(name="spool", bufs=2)
        uvpool = tc.alloc_tile_pool(name="uvpool", bufs=2)

        # m index mapping everywhere: m = p * MT + t  (p on partitions)
        for b in range(BL):
            # rank-2 operands: U_b = [ones; coverage[b]] (strided views per
            # tile), V_b = [dec[b]; wc]
            U_b = uvpool.tile([2, M], enc_dt, name="U")
            nc.gpsimd.dma_start(out=U_b[0:1, :], in_=ones.unsqueeze(0))
            nc.gpsimd.dma_start(out=U_b[1:2, :], in_=cov[b:b + 1, :])
            Uv = U_b.rearrange("k (p t) -> k t p", t=MT)
            V_b = uvpool.tile([2, N], enc_dt, name="V")
            nc.gpsimd.dma_start(out=V_b[1:2, :], in_=wc.unsqueeze(0))
            nc.gpsimd.dma_start(out=V_b[0:1, :], in_=dec_sb[b:b + 1, :])

            # phase 1: att -> tanh -> scores
            scores_b = spool.tile([P, MT], f32, name="scores")
            for g in range(MT // MG_F):
                encf_t = fpool.tile([P, MG_F, N], enc_dt)
                src = encf[b * M:(b + 1) * M, :].rearrange(
                    "(p t) n -> p t n", t=MT)[:, g * MG_F:(g + 1) * MG_F, :]
                nc.sync.dma_start(out=encf_t, in_=src)
                for t in range(MG_F):
                    mt = g * MG_F + t
                    att_ps = apool.tile([P, N], f32, name="att")
                    for h in range(NHALF):
                        sl = slice(h * 512, (h + 1) * 512)
                        nc.tensor.matmul(
                            att_ps[:, sl], lhsT=Uv[:, mt, :], rhs=V_b[:, sl],
                            start=True, stop=True,
                        )
                    att_sl = encf_t[:, t, :].bitcast(f32)
                    nc.vector.scalar_tensor_tensor(
                        out=att_sl, in0=att_sl, scalar=1.0, in1=att_ps,
                        op0=Alu.mult, op1=Alu.add,
                    )
                    tanh_t = tpool.tile([P, N], f32, name="tanh")
                    nc.scalar.activation(tanh_t, att_sl, Act.Tanh)
                    nc.vector.scalar_tensor_tensor(
                        out=tanh_t, in0=tanh_t, scalar=1.0, in1=v_bc,
                        op0=Alu.mult, op1=Alu.mult,
                        accum_out=scores_b[:, mt:mt + 1],
                    )

            # phase 2: masked softmax over m
            exp_t = spool.tile([P, MT], f32, name="exp")
            nc.scalar.activation(exp_t, scores_b, Act.Exp)
            mask_t = spool.tile([P, MT], f32, name="mask_t")
            nc.gpsimd.dma_start(out=mask_t, in_=mask[b, :].rearrange("(p t) -> p t", t=MT))
            om_t = spool.tile([P, MT], f32, name="om")
            nc.vector.tensor_scalar(om_t, mask_t, -1.0, 1.0, Alu.mult, Alu.add)
            rsum = spool.tile([P, 1], f32, name="rsum")
            me_t = spool.tile([P, MT], f32, name="me")
            nc.vector.scalar_tensor_tensor(
                out=me_t, in0=exp_t, scalar=1.0, in1=om_t,
                op0=Alu.mult, op1=Alu.mult, accum_out=rsum,
            )
            tot_b = spool.tile([P, 1], f32, name="tot_b")
            nc.gpsimd.partition_all_reduce(
                tot_b, rsum, channels=P, reduce_op=bass_isa.ReduceOp.add
            )
            rb = spool.tile([P, 1], f32, name="rb")
            nc.vector.reciprocal(rb, tot_b)
            attn_t = spool.tile([P, MT], f32, name="attn_t")
            nc.vector.tensor_scalar_mul(attn_t, me_t, rb)
            cov_t = spool.tile([P, MT], enc_dt, name="cov_t")
            nc.gpsimd.dma_start(out=cov_t, in_=cov[b, :].rearrange("(p t) -> p t", t=MT))
            ncov_t = spool.tile([P, MT], f32, name="ncov_t")
            nc.vector.tensor_add(ncov_t, cov_t, attn_t)
            nc.gpsimd.dma_start(
                out=attn_out[b, :].rearrange("(p t) -> p t", t=MT), in_=attn_t
            )
            nc.gpsimd.dma_start(
                out=ncov_out[b, :].rearrange("(p t) -> p t", t=MT), in_=ncov_t
            )

            # phase 3: context = attn @ enc_outputs[b]
            attn_r = spool.tile([P, MT], ctx_dt, name="attn_r")
            nc.vector.tensor_copy(attn_r, attn_t)
            ctx_ps = cpool.tile([1, N], f32, tag="ctx", name="ctx_ps")
            for g in range(MT // MG_O):
                enco_t = opool.tile([P, MG_O, N], ctx_dt)
                src = enco[b, :, :].rearrange(
                    "(p t) n -> p t n", t=MT)[:, g * MG_O:(g + 1) * MG_O, :]
                nc.scalar.dma_start(out=enco_t, in_=src)
                for t in range(MG_O):
                    mt = g * MG_O + t
                    for h in range(NHALF):
                        sl = slice(h * 512, (h + 1) * 512)
                        nc.tensor.matmul(
                            ctx_ps[0:1, sl],
                            lhsT=attn_r[:, mt:mt + 1],
                            rhs=enco_t[:, t, sl],
                            start=(mt == 0),
                            stop=(mt == MT - 1),
                        )
            ctx_sb = spool.tile([1, N], f32, name="ctx_sb")
            nc.scalar.copy(ctx_sb, ctx_ps)
            nc.gpsimd.dma_start(out=ctx_out[b:b + 1, :], in_=ctx_sb)

        uvpool.release()
        spool.release()
        tpool.release()
        opool.release()
        fpool.release()
        cpool.release()
        apool.release()
        consts.release()

    nc.finalize()
    return nc


def kernel(h_c_hat, enc_outputs, enc_feature, enc_padding_mask, coverage,
           Wd, bd, wc, v):
    global LAST_RESULT
    from concourse.bass_utils import run_bass_kernel_spmd

    if "nc" not in _CACHE:
        _CACHE["nc"] = _build_nc()
    nc = _CACHE["nc"]

    f = np.float32
    WdT = np.ascontiguousarray(np.asarray(Wd, dtype=f).T)
    hT_full = np.ascontiguousarray(np.asarray(h_c_hat, dtype=f).T)  # [N, B]
    enc_feature = np.asarray(enc_feature, dtype=f).reshape(B, M, N)
    enc_outputs = np.asarray(enc_outputs, dtype=f)
    enc_padding_mask = np.asarray(enc_padding_mask, dtype=f)
    coverage = np.asarray(coverage, dtype=f)
    bd = np.ascontiguousarray(np.asarray(bd, dtype=f))
    wc = np.ascontiguousarray(np.asarray(wc, dtype=f))
    v = np.ascontiguousarray(np.asarray(v, dtype=f))

    in_maps = []
    for c in range(NCORES):
        s = slice(c * BL, (c + 1) * BL)
        in_maps.append({
            "enc_feature": np.ascontiguousarray(
                enc_feature[s].reshape(BL * M, N)),
            "enc_outputs": np.ascontiguousarray(enc_outputs[s]),
            "h_T": np.ascontiguousarray(hT_full[:, s]),
            "WdT": WdT,
            "bd": bd,
            "wc": wc,
            "v": v,
            "coverage": np.ascontiguousarray(coverage[s]),
            "mask": np.ascontiguousarray(enc_padding_mask[s]),
            "ones": np.ones(M, dtype=f),
        })

    trace = bool(os.environ.get("BASS_TRACE"))
    tmpdir = os.environ.get("BASS_KERNEL_TMPDIR") or None
    LAST_RESULT = run_bass_kernel_spmd(
        nc, in_maps, core_ids=list(range(NCORES)), trace=trace, tmpdir=tmpdir
    )
    res = LAST_RESULT.results

    context_vec = np.concatenate([res[c]["context_vec"] for c in range(NCORES)], 0)
    attn = np.concatenate([res[c]["attn"] for c in range(NCORES)], 0)
    new_coverage = np.concatenate([res[c]["new_coverage"] for c in range(NCORES)], 0)
    return context_vec, attn, new_coverage


# revision 26
# speedup vs baseline: 1.0816x; 1.0318x over previous
"""Trainium2 Bass kernel for the coverage-attention module.

Computes, for inputs h_c_hat [B,N], enc_outputs [B,M,N], enc_feature [B*M,N],
enc_padding_mask [B,M], coverage [B,M] and params Wd [N,N], bd, wc, v [N]:

    dec      = h_c_hat @ Wd.T + bd                      # [B,N]
    att      = enc_feature + dec[:,None,:] + coverage[:,:,None]*wc
    scores   = tanh(att) @ v                            # [B,M]
    attn     = softmax(scores) * (1-mask); attn /= attn.sum(-1)
    context  = einsum('bm,bmn->bn', attn, enc_outputs)
    new_cov  = coverage + attn

Sharding: data-parallel over batch across 8 NeuronCores (4 batches/core),
params replicated. Inside each core everything stays in the natural
[m-on-partitions, n-on-free] layout:
  * rank-2 term ones*dec + cov*wc is a K=2 matmul accumulated in PSUM on top
    of an identity-matmul copy of enc_feature (float32r, 1 cyc/row),
  * tanh on ScalarE reads PSUM directly,
  * scores' n-contraction is the fused DVE tensor_tensor_reduce (accum_out),
  * softmax's m-reduction is a tiny GPSIMD partition reduce,
  * context matmul contracts m on partitions (natural layout).
Host-side prep is layout-only: Wd and h_c_hat are passed pre-transposed.
"""

import os

import numpy as np

B, M, N = 32, 2048, 1024
NCORES = 8
BL = B // NCORES          # batches per core
P = 128                   # partitions
MT = M // P               # m-tiles per batch
MG_F = 4                  # m-tiles per enc_feature DMA group
MG_O = 4                  # m-tiles per enc_outputs DMA group
NHALF = N // 512

_CACHE = {}
LAST_RESULT = None        # BassKernelResults of the most recent run (for test.py)

# float32r (single-pass reduced-precision fp32 matmul) for the bulk matmuls;
# flip to exact fp32 (4 cyc/row) if precision ever demands it.
USE_F32R_ENC = True
USE_F32R_CTX = True


def _build_nc():
    import concourse.bass as bass
    import concourse.tile as tile
    from concourse import bacc, bass_isa, library_config, mybir

    f32 = mybir.dt.float32
    f32r = mybir.dt.float32r
    Alu = mybir.AluOpType
    Act = mybir.ActivationFunctionType

    enc_dt = f32r if USE_F32R_ENC else f32
    ctx_dt = f32r if USE_F32R_CTX else f32

    nc = bacc.Bacc("TRN2", target_bir_lowering=False, debug=False)

    encf = nc.dram_tensor("enc_feature", [BL * M, N], enc_dt, kind="ExternalInput").ap()
    enco = nc.dram_tensor("enc_outputs", [BL, M, N], ctx_dt, kind="ExternalInput").ap()
    hT = nc.dram_tensor("h_T", [N, BL], enc_dt, kind="ExternalInput").ap()
    WdT = nc.dram_tensor("WdT", [N, N], enc_dt, kind="ExternalInput").ap()
    bd = nc.dram_tensor("bd", [N], f32, kind="ExternalInput").ap()
    wc = nc.dram_tensor("wc", [N], enc_dt, kind="ExternalInput").ap()
    v = nc.dram_tensor("v", [N], f32, kind="ExternalInput").ap()
    cov = nc.dram_tensor("coverage", [BL, M], enc_dt, kind="ExternalInput").ap()
    mask = nc.dram_tensor("mask", [BL, M], f32, kind="ExternalInput").ap()
    ones = nc.dram_tensor("ones", [M], enc_dt, kind="ExternalInput").ap()
    ctx_out = nc.dram_tensor("context_vec", [BL, N], f32, kind="ExternalOutput").ap()
    attn_out = nc.dram_tensor("attn", [BL, M], f32, kind="ExternalOutput").ap()
    ncov_out = nc.dram_tensor("new_coverage", [BL, M], f32, kind="ExternalOutput").ap()

    def pbcast(ap, parts):
        # 0-stride partition broadcast of a 1-partition source AP (DRAM src only)
        return bass.AP(tensor=ap.tensor, offset=ap.offset, ap=[[0, parts], *ap.ap])

    with tile.TileContext(nc) as tc:
        consts = tc.alloc_tile_pool(name="consts", bufs=1)
        apool = tc.alloc_tile_pool(name="apool", bufs=3, space="PSUM")
        cpool = tc.alloc_tile_pool(name="cpool", bufs=1, space="PSUM")
        setup = tc.alloc_tile_pool(name="setup", bufs=1)

        nc.gpsimd.load_library(library_config.attn)

        bf16 = mybir.dt.bfloat16
        v_bc = consts.tile([P, N], bf16)
        nc.gpsimd.dma_start(out=v_bc, in_=pbcast(v.unsqueeze(0), P))

        # ---- dec = h @ Wd.T + bd, full fp32 (setup; WdT freed afterwards) ----
        WdT_sb = setup.tile([P, N // P, N], enc_dt)
        WdT_r = WdT.rearrange("(c p) i -> p c i", p=P)
        hT_sb = setup.tile([P, N // P, BL], enc_dt)
        nc.scalar.dma_start(out=hT_sb, in_=hT.rearrange("(c p) b -> p c b", p=P))
        bd_bc = setup.tile([BL, N], f32)
        nc.gpsimd.dma_start(out=bd_bc, in_=pbcast(bd.unsqueeze(0), BL))

        dec_ps = cpool.tile([BL, N], f32, tag="ctx")
        for jc in range(N // P):
            nc.scalar.dma_start(out=WdT_sb[:, jc, :], in_=WdT_r[:, jc, :])
            for h in range(NHALF):
                nc.tensor.matmul(
                    dec_ps[:, h * 512:(h + 1) * 512],
                    lhsT=hT_sb[:, jc, :],
                    rhs=WdT_sb[:, jc, h * 512:(h + 1) * 512],
                    start=(jc == 0),
                    stop=(jc == N // P - 1),
                )
        dec_sb = consts.tile([BL, N], enc_dt)
        nc.vector.tensor_add(dec_sb, dec_ps, bd_bc)
        setup.release()

        fpool = tc.alloc_tile_pool(name="fpool", bufs=4)
        opool = tc.alloc_tile_pool(name="opool", bufs=4)
        tpool = tc.alloc_tile_pool(name="tpool", bufs=3)
        spool = tc.alloc_tile_pool(name="spool", bufs=2)
        uvpool = tc.alloc_tile_pool(name="uvpool", bufs=2)

        # m index mapping everywhere: m = p * MT + t  (p on partitions)
        for b in range(BL):
            # rank-2 operands: U_b = [ones; coverage[b]] (strided views per
            # tile), V_b = [dec[b]; wc]
            U_b = uvpool.tile([2, M], enc_dt, name="U")
            nc.gpsimd.dma_start(out=U_b[0:1, :], in_=ones.unsqueeze(0))
            nc.gpsimd.dma_start(out=U_b[1:2, :], in_=cov[b:b + 1, :])
            Uv = U_b.rearrange("k (p t) -> k t p", t=MT)
            V_b = uvpool.tile([2, N], enc_dt, name="V")
            nc.gpsimd.dma_start(out=V_b[1:2, :], in_=wc.unsqueeze(0))
            nc.gpsimd.dma_start(out=V_b[0:1, :], in_=dec_sb[b:b + 1, :])

            # phase 1: att -> tanh -> scores
            scores_b = spool.tile([P, MT], f32, name="scores")
            for g in range(MT // MG_F):
                encf_t = fpool.tile([P, MG_F, N], enc_dt)
                src = encf[b * M:(b + 1) * M, :].rearrange(
                    "(p t) n -> p t n", t=MT)[:, g * MG_F:(g + 1) * MG_F, :]
                nc.sync.dma_start(out=encf_t, in_=src)
                for t in range(MG_F):
                    mt = g * MG_F + t
                    att_ps = apool.tile([P, N], f32, name="att")
                    for h in range(NHALF):
                        sl = slice(h * 512, (h + 1) * 512)
                        nc.tensor.matmul(
                            att_ps[:, sl], lhsT=Uv[:, mt, :], rhs=V_b[:, sl],
                            start=True, stop=True,
                        )
                    att_sl = encf_t[:, t, :].bitcast(f32)
                    nc.vector.scalar_tensor_tensor(
                        out=att_sl, in0=att_sl, scalar=1.0, in1=att_ps,
                        op0=Alu.mult, op1=Alu.add,
                    )
                    tanh_t = tpool.tile([P, N], bf16, name="tanh")
                    nc.scalar.activation(tanh_t, att_sl, Act.Tanh)
                    nc.vector.scalar_tensor_tensor(
                        out=tanh_t, in0=tanh_t, scalar=1.0, in1=v_bc,
                        op0=Alu.mult, op1=Alu.mult,
                        accum_out=scores_b[:, mt:mt + 1],
                    )

            # phase 2: masked softmax over m
            exp_t = spool.tile([P, MT], f32, name="exp")
            nc.scalar.activation(exp_t, scores_b, Act.Exp)
            mask_t = spool.tile([P, MT], f32, name="mask_t")
            nc.gpsimd.dma_start(out=mask_t, in_=mask[b, :].rearrange("(p t) -> p t", t=MT))
            om_t = spool.tile([P, MT], f32, name="om")
            nc.vector.tensor_scalar(om_t, mask_t, -1.0, 1.0, Alu.mult, Alu.add)
            rsum = spool.tile([P, 1], f32, name="rsum")
            me_t = spool.tile([P, MT], f32, name="me")
            nc.vector.scalar_tensor_tensor(
                out=me_t, in0=exp_t, scalar=1.0, in1=om_t,
                op0=Alu.mult, op1=Alu.mult, accum_out=rsum,
            )
            tot_b = spool.tile([P, 1], f32, name="tot_b")
            nc.gpsimd.partition_all_reduce(
                tot_b, rsum, channels=P, reduce_op=bass_isa.ReduceOp.add
            )
            rb = spool.tile([P, 1], f32, name="rb")
            nc.vector.reciprocal(rb, tot_b)
            attn_t = spool.tile([P, MT], f32, name="attn_t")
            nc.vector.tensor_scalar_mul(attn_t, me_t, rb)
            cov_t = spool.tile([P, MT], enc_dt, name="cov_t")
            nc.gpsimd.dma_start(out=cov_t, in_=cov[b, :].rearrange("(p t) -> p t", t=MT))
            ncov_t = spool.tile([P, MT], f32, name="ncov_t")
            nc.vector.tensor_add(ncov_t, cov_t, attn_t)
            nc.gpsimd.dma_start(
                out=attn_out[b, :].rearrange("(p t) -> p t", t=MT), in_=attn_t
            )
            nc.gpsimd.dma_start(
                out=ncov_out[b, :].rearrange("(p t) -> p t", t=MT), in_=ncov_t
            )

            # phase 3: context = attn @ enc_outputs[b]
            attn_r = spool.tile([P, MT], ctx_dt, name="attn_r")
            nc.vector.tensor_copy(attn_r, attn_t)
            ctx_ps = cpool.tile([1, N], f32, tag="ctx", name="ctx_ps")
            for g in range(MT // MG_O):
                enco_t = opool.tile([P, MG_O, N], ctx_dt)
                src = enco[b, :, :].rearrange(
                    "(p t) n -> p t n", t=MT)[:, g * MG_O:(g + 1) * MG_O, :]
                nc.sync.dma_start(out=enco_t, in_=src)
                for t in range(MG_O):
                    mt = g * MG_O + t
                    for h in range(NHALF):
                        sl = slice(h * 512, (h + 1) * 512)
                        nc.tensor.matmul(
                            ctx_ps[0:1, sl],
                            lhsT=attn_r[:, mt:mt + 1],
                            rhs=enco_t[:, t, sl],
                            start=(mt == 0),
                            stop=(mt == MT - 1),
                        )
            ctx_sb = spool.tile([1, N], f32, name="ctx_sb")
            nc.scalar.copy(ctx_sb, ctx_ps)
            nc.gpsimd.dma_start(out=ctx_out[b:b + 1, :], in_=ctx_sb)

        uvpool.release()
        spool.release()
        tpool.release()
        opool.release()
        fpool.release()
        cpool.release()
        apool.release()
        consts.release()

    nc.finalize()
    return nc


def kernel(h_c_hat, enc_outputs, enc_feature, enc_padding_mask, coverage,
           Wd, bd, wc, v):
    global LAST_RESULT
    from concourse.bass_utils import run_bass_kernel_spmd

    if "nc" not in _CACHE:
        _CACHE["nc"] = _build_nc()
    nc = _CACHE["nc"]

    f = np.float32
    WdT = np.ascontiguousarray(np.asarray(Wd, dtype=f).T)
    hT_full = np.ascontiguousarray(np.asarray(h_c_hat, dtype=f).T)  # [N, B]
    enc_feature = np.asarray(enc_feature, dtype=f).reshape(B, M, N)
    enc_outputs = np.asarray(enc_outputs, dtype=f)
    enc_padding_mask = np.asarray(enc_padding_mask, dtype=f)
    coverage = np.asarray(coverage, dtype=f)
    bd = np.ascontiguousarray(np.asarray(bd, dtype=f))
    wc = np.ascontiguousarray(np.asarray(wc, dtype=f))
    v = np.ascontiguousarray(np.asarray(v, dtype=f))

    in_maps = []
    for c in range(NCORES):
        s = slice(c * BL, (c + 1) * BL)
        in_maps.append({
            "enc_feature": np.ascontiguousarray(
                enc_feature[s].reshape(BL * M, N)),
            "enc_outputs": np.ascontiguousarray(enc_outputs[s]),
            "h_T": np.ascontiguousarray(hT_full[:, s]),
            "WdT": WdT,
            "bd": bd,
            "wc": wc,
            "v": v,
            "coverage": np.ascontiguousarray(coverage[s]),
            "mask": np.ascontiguousarray(enc_padding_mask[s]),
            "ones": np.ones(M, dtype=f),
        })

    trace = bool(os.environ.get("BASS_TRACE"))
    tmpdir = os.environ.get("BASS_KERNEL_TMPDIR") or None
    LAST_RESULT = run_bass_kernel_spmd(
        nc, in_maps, core_ids=list(range(NCORES)), trace=trace, tmpdir=tmpdir
    )
    res = LAST_RESULT.results

    context_vec = np.concatenate([res[c]["context_vec"] for c in range(NCORES)], 0)
    attn = np.concatenate([res[c]["attn"] for c in range(NCORES)], 0)
    new_coverage = np.concatenate([res[c]["new_coverage"] for c in range(NCORES)], 0)
    return context_vec, attn, new_coverage


# revision 28
# speedup vs baseline: 1.1438x; 1.0575x over previous
"""Trainium2 Bass kernel for the coverage-attention module.

Computes, for inputs h_c_hat [B,N], enc_outputs [B,M,N], enc_feature [B*M,N],
enc_padding_mask [B,M], coverage [B,M] and params Wd [N,N], bd, wc, v [N]:

    dec      = h_c_hat @ Wd.T + bd                      # [B,N]
    att      = enc_feature + dec[:,None,:] + coverage[:,:,None]*wc
    scores   = tanh(att) @ v                            # [B,M]
    attn     = softmax(scores) * (1-mask); attn /= attn.sum(-1)
    context  = einsum('bm,bmn->bn', attn, enc_outputs)
    new_cov  = coverage + attn

Sharding: data-parallel over batch across 8 NeuronCores (4 batches/core),
params replicated. Inside each core everything stays in the natural
[m-on-partitions, n-on-free] layout:
  * rank-2 term ones*dec + cov*wc is a K=2 matmul accumulated in PSUM on top
    of an identity-matmul copy of enc_feature (float32r, 1 cyc/row),
  * tanh on ScalarE reads PSUM directly,
  * scores' n-contraction is the fused DVE tensor_tensor_reduce (accum_out),
  * softmax's m-reduction is a tiny GPSIMD partition reduce,
  * context matmul contracts m on partitions (natural layout).
Host-side prep is layout-only: Wd and h_c_hat are passed pre-transposed.
"""

import os

import numpy as np

B, M, N = 32, 2048, 1024
NCORES = 8
BL = B // NCORES          # batches per core
P = 128                   # partitions
MT = M // P               # m-tiles per batch
MG_F = 4                  # m-tiles per enc_feature DMA group
MG_O = 4                  # m-tiles per enc_outputs DMA group
NHALF = N // 512

_CACHE = {}
LAST_RESULT = None        # BassKernelResults of the most recent run (for test.py)

# float32r (single-pass reduced-precision fp32 matmul) for the bulk matmuls;
# flip to exact fp32 (4 cyc/row) if precision ever demands it.
USE_F32R_ENC = True
USE_F32R_CTX = True


def _build_nc():
    import concourse.bass as bass
    import concourse.tile as tile
    from concourse import bacc, bass_isa, library_config, mybir

    f32 = mybir.dt.float32
    f32r = mybir.dt.float32r
    Alu = mybir.AluOpType
    Act = mybir.ActivationFunctionType

    enc_dt = f32r if USE_F32R_ENC else f32
    ctx_dt = f32r if USE_F32R_CTX else f32

    nc = bacc.Bacc("TRN2", target_bir_lowering=False, debug=False)

    encf = nc.dram_tensor("enc_feature", [BL * M, N], enc_dt, kind="ExternalInput").ap()
    enco = nc.dram_tensor("enc_outputs", [BL, M, N], ctx_dt, kind="ExternalInput").ap()
    hT = nc.dram_tensor("h_T", [N, BL], enc_dt, kind="ExternalInput").ap()
    WdT = nc.dram_tensor("WdT", [N, N], enc_dt, kind="ExternalInput").ap()
    bd = nc.dram_tensor("bd", [N], f32, kind="ExternalInput").ap()
    wc = nc.dram_tensor("wc", [N], enc_dt, kind="ExternalInput").ap()
    v = nc.dram_tensor("v", [N], f32, kind="ExternalInput").ap()
    cov = nc.dram_tensor("coverage", [BL, M], enc_dt, kind="ExternalInput").ap()
    mask = nc.dram_tensor("mask", [BL, M], f32, kind="ExternalInput").ap()
    ones = nc.dram_tensor("ones", [M], enc_dt, kind="ExternalInput").ap()
    ident = nc.dram_tensor("identity", [P, P], enc_dt, kind="ExternalInput").ap()
    ctx_out = nc.dram_tensor("context_vec", [BL, N], f32, kind="ExternalOutput").ap()
    attn_out = nc.dram_tensor("attn", [BL, M], f32, kind="ExternalOutput").ap()
    ncov_out = nc.dram_tensor("new_coverage", [BL, M], f32, kind="ExternalOutput").ap()

    def pbcast(ap, parts):
        # 0-stride partition broadcast of a 1-partition source AP (DRAM src only)
        return bass.AP(tensor=ap.tensor, offset=ap.offset, ap=[[0, parts], *ap.ap])

    with tile.TileContext(nc) as tc:
        consts = tc.alloc_tile_pool(name="consts", bufs=1)
        apool = tc.alloc_tile_pool(name="apool", bufs=3, space="PSUM")
        cpool = tc.alloc_tile_pool(name="cpool", bufs=1, space="PSUM")
        setup = tc.alloc_tile_pool(name="setup", bufs=1)

        nc.gpsimd.load_library(library_config.attn)

        id_sb = consts.tile([P, P], enc_dt)
        nc.scalar.dma_start(out=id_sb, in_=ident)
        v_bc = consts.tile([P, N], f32)
        nc.gpsimd.dma_start(out=v_bc, in_=pbcast(v.unsqueeze(0), P))

        # ---- dec = h @ Wd.T + bd, full fp32 (setup; WdT freed afterwards) ----
        WdT_sb = setup.tile([P, N // P, N], enc_dt)
        WdT_r = WdT.rearrange("(c p) i -> p c i", p=P)
        hT_sb = setup.tile([P, N // P, BL], enc_dt)
        nc.scalar.dma_start(out=hT_sb, in_=hT.rearrange("(c p) b -> p c b", p=P))
        bd_bc = setup.tile([BL, N], f32)
        nc.gpsimd.dma_start(out=bd_bc, in_=pbcast(bd.unsqueeze(0), BL))

        dec_ps = cpool.tile([BL, N], f32, tag="ctx")
        for jc in range(N // P):
            nc.scalar.dma_start(out=WdT_sb[:, jc, :], in_=WdT_r[:, jc, :])
            for h in range(NHALF):
                nc.tensor.matmul(
                    dec_ps[:, h * 512:(h + 1) * 512],
                    lhsT=hT_sb[:, jc, :],
                    rhs=WdT_sb[:, jc, h * 512:(h + 1) * 512],
                    start=(jc == 0),
                    stop=(jc == N // P - 1),
                )
        dec_sb = consts.tile([BL, N], enc_dt)
        nc.vector.tensor_add(dec_sb, dec_ps, bd_bc)
        setup.release()

        fpool = tc.alloc_tile_pool(name="fpool", bufs=4)
        opool = tc.alloc_tile_pool(name="opool", bufs=4)
        tpool = tc.alloc_tile_pool(name="tpool", bufs=3)
        spool = tc.alloc_tile_pool(name="spool", bufs=2)
        uvpool = tc.alloc_tile_pool(name="uvpool", bufs=2)

        # m index mapping everywhere: m = p * MT + t  (p on partitions)
        for b in range(BL):
            # rank-2 operands: U_b = [ones; coverage[b]] (strided views per
            # tile), V_b = [dec[b]; wc]
            U_b = uvpool.tile([2, M], enc_dt, name="U")
            nc.gpsimd.dma_start(out=U_b[0:1, :], in_=ones.unsqueeze(0))
            nc.gpsimd.dma_start(out=U_b[1:2, :], in_=cov[b:b + 1, :])
            Uv = U_b.rearrange("k (p t) -> k t p", t=MT)
            V_b = uvpool.tile([2, N], enc_dt, name="V")
            nc.gpsimd.dma_start(out=V_b[1:2, :], in_=wc.unsqueeze(0))
            nc.gpsimd.dma_start(out=V_b[0:1, :], in_=dec_sb[b:b + 1, :])

            # phase 1: att -> tanh -> scores
            scores_b = spool.tile([P, MT], f32, name="scores")
            for g in range(MT // MG_F):
                encf_t = fpool.tile([P, MG_F, N], enc_dt)
                src = encf[b * M:(b + 1) * M, :].rearrange(
                    "(p t) n -> p t n", t=MT)[:, g * MG_F:(g + 1) * MG_F, :]
                nc.sync.dma_start(out=encf_t, in_=src)
                for t in range(MG_F):
                    mt = g * MG_F + t
                    att_ps = apool.tile([P, N], f32, name="att")
                    on_pe = (mt % 2 == 0)
                    for h in range(NHALF):
                        sl = slice(h * 512, (h + 1) * 512)
                        if on_pe:
                            nc.tensor.matmul(
                                att_ps[:, sl], lhsT=id_sb, rhs=encf_t[:, t, sl],
                                start=True, stop=False,
                            )
                        nc.tensor.matmul(
                            att_ps[:, sl], lhsT=Uv[:, mt, :], rhs=V_b[:, sl],
                            start=not on_pe, stop=True,
                        )
                    if on_pe:
                        tanh_in = att_ps
                    else:
                        att_t = tpool.tile([P, N], f32, name="att_t")
                        nc.vector.scalar_tensor_tensor(
                            out=att_t, in0=encf_t[:, t, :], scalar=1.0,
                            in1=att_ps, op0=Alu.mult, op1=Alu.add,
                        )
                        tanh_in = att_t
                    tanh_t = tpool.tile([P, N], f32, name="tanh")
                    nc.scalar.activation(tanh_t, tanh_in, Act.Tanh)
                    nc.vector.scalar_tensor_tensor(
                        out=tanh_t, in0=tanh_t, scalar=1.0, in1=v_bc,
                        op0=Alu.mult, op1=Alu.mult,
                        accum_out=scores_b[:, mt:mt + 1],
                    )

            # phase 2: masked softmax over m
            exp_t = spool.tile([P, MT], f32, name="exp")
            nc.scalar.activation(exp_t, scores_b, Act.Exp)
            mask_t = spool.tile([P, MT], f32, name="mask_t")
            nc.gpsimd.dma_start(out=mask_t, in_=mask[b, :].rearrange("(p t) -> p t", t=MT))
            om_t = spool.tile([P, MT], f32, name="om")
            nc.vector.tensor_scalar(om_t, mask_t, -1.0, 1.0, Alu.mult, Alu.add)
            rsum = spool.tile([P, 1], f32, name="rsum")
            me_t = spool.tile([P, MT], f32, name="me")
            nc.vector.scalar_tensor_tensor(
                out=me_t, in0=exp_t, scalar=1.0, in1=om_t,
                op0=Alu.mult, op1=Alu.mult, accum_out=rsum,
            )
            tot_b = spool.tile([P, 1], f32, name="tot_b")
            nc.gpsimd.partition_all_reduce(
                tot_b, rsum, channels=P, reduce_op=bass_isa.ReduceOp.add
            )
            rb = spool.tile([P, 1], f32, name="rb")
            nc.vector.reciprocal(rb, tot_b)
            attn_t = spool.tile([P, MT], f32, name="attn_t")
            nc.vector.tensor_scalar_mul(attn_t, me_t, rb)
            cov_t = spool.tile([P, MT], enc_dt, name="cov_t")
            nc.gpsimd.dma_start(out=cov_t, in_=cov[b, :].rearrange("(p t) -> p t", t=MT))
            ncov_t = spool.tile([P, MT], f32, name="ncov_t")
            nc.vector.tensor_add(ncov_t, cov_t, attn_t)
            nc.gpsimd.dma_start(
                out=attn_out[b, :].rearrange("(p t) -> p t", t=MT), in_=attn_t
            )
            nc.gpsimd.dma_start(
                out=ncov_out[b, :].rearrange("(p t) -> p t", t=MT), in_=ncov_t
            )

            # phase 3: context = attn @ enc_outputs[b]
            attn_r = spool.tile([P, MT], ctx_dt, name="attn_r")
            nc.vector.tensor_copy(attn_r, attn_t)
            ctx_ps = cpool.tile([1, N], f32, tag="ctx", name="ctx_ps")
            for g in range(MT // MG_O):
                enco_t = opool.tile([P, MG_O, N], ctx_dt)
                src = enco[b, :, :].rearrange(
                    "(p t) n -> p t n", t=MT)[:, g * MG_O:(g + 1) * MG_O, :]
                nc.sync.dma_start(out=enco_t, in_=src)
                for t in range(MG_O):
                    mt = g * MG_O + t
                    for h in range(NHALF):
                        sl = slice(h * 512, (h + 1) * 512)
                        nc.tensor.matmul(
                            ctx_ps[0:1, sl],
                            lhsT=attn_r[:, mt:mt + 1],
                            rhs=enco_t[:, t, sl],
                            start=(mt == 0),
                            stop=(mt == MT - 1),
                        )
            ctx_sb = spool.tile([1, N], f32, name="ctx_sb")
            nc.scalar.copy(ctx_sb, ctx_ps)
            nc.gpsimd.dma_start(out=ctx_out[b:b + 1, :], in_=ctx_sb)

        uvpool.release()
        spool.release()
        tpool.release()
        opool.release()
        fpool.release()
        cpool.release()
        apool.release()
        consts.release()

    nc.finalize()
    return nc


def kernel(h_c_hat, enc_outputs, enc_feature, enc_padding_mask, coverage,
           Wd, bd, wc, v):
    global LAST_RESULT
    from concourse.bass_utils import run_bass_kernel_spmd

    if "nc" not in _CACHE:
        _CACHE["nc"] = _build_nc()
    nc = _CACHE["nc"]

    f = np.float32
    WdT = np.ascontiguousarray(np.asarray(Wd, dtype=f).T)
    hT_full = np.ascontiguousarray(np.asarray(h_c_hat, dtype=f).T)  # [N, B]
    enc_feature = np.asarray(enc_feature, dtype=f).reshape(B, M, N)
    enc_outputs = np.asarray(enc_outputs, dtype=f)
    enc_padding_mask = np.asarray(enc_padding_mask, dtype=f)
    coverage = np.asarray(coverage, dtype=f)
    bd = np.ascontiguousarray(np.asarray(bd, dtype=f))
    wc = np.ascontiguousarray(np.asarray(wc, dtype=f))
    v = np.ascontiguousarray(np.asarray(v, dtype=f))

    in_maps = []
    for c in range(NCORES):
        s = slice(c * BL, (c + 1) * BL)
        in_maps.append({
            "enc_feature": np.ascontiguousarray(
                enc_feature[s].reshape(BL * M, N)),
            "enc_outputs": np.ascontiguousarray(enc_outputs[s]),
            "h_T": np.ascontiguousarray(hT_full[:, s]),
            "WdT": WdT,
            "bd": bd,
            "wc": wc,
            "v": v,
            "coverage": np.ascontiguousarray(coverage[s]),
            "mask": np.ascontiguousarray(enc_padding_mask[s]),
            "ones": np.ones(M, dtype=f),
            "identity": np.eye(P, dtype=f),
        })

    trace = bool(os.environ.get("BASS_TRACE"))
    tmpdir = os.environ.get("BASS_KERNEL_TMPDIR") or None
    LAST_RESULT = run_bass_kernel_spmd(
        nc, in_maps, core_ids=list(range(NCORES)), trace=trace, tmpdir=tmpdir
    )
    res = LAST_RESULT.results

    context_vec = np.concatenate([res[c]["context_vec"] for c in range(NCORES)], 0)
    attn = np.concatenate([res[c]["attn"] for c in range(NCORES)], 0)
    new_coverage = np.concatenate([res[c]["new_coverage"] for c in range(NCORES)], 0)
    return context_vec, attn, new_coverage
